# revision 4
# baseline (speedup 1.0000x reference)
"""Per-channel EMA (first-order linear recurrence along time) on 8 TRN2 cores.

  y[b, c, 0] = x[b, c, 0]
  y[b, c, t] = (1 - alpha[c]) * y[b, c, t-1] + alpha[c] * x[b, c, t]

Strategy
  - Data-parallel over batch: B=32 -> 4 batches per core, alpha replicated.
  - Per core: 16 tiles of [128 channels (partitions), 2048 time (free)].
  - The recurrence runs at line rate on the DVE via tensor_tensor_scan:
        state = (d * state) + a*x_t,   d = 1 - alpha (per partition)
    with initial = x[:, 0] as a per-partition AP, so y[:, 0] = x[:, 0] is
    handled by a 1-column copy and the scan covers columns 1..L-1.
  - The alpha pre-scale (a*x) runs on the Scalar/ACT engine so DVE and ACT
    each make one pass per tile and both hide behind the HBM DMA (memory
    bound: 32 MiB per core round trip).
"""

import numpy as np

import concourse.bass as bass
import concourse.bacc as bacc
import concourse.mybir as mybir
from concourse.tile import TileContext
from concourse.bass_utils import run_bass_kernel_spmd

B, C, L = 32, 512, 2048
N_CORES = 8
B_SH = B // N_CORES  # 4 batches per core
P = 128              # SBUF partitions
N_CB = C // P        # 4 channel blocks

_F32 = mybir.dt.float32


def build_nc() -> bass.Bass:
    # Bacc (not raw Bass): its compile() runs generate_event_semaphores,
    # which splits multi-sem waits — TRN2 allows at most one wait command
    # per instruction, and Tile freely emits several.
    nc = bacc.Bacc()
    x = nc.dram_tensor("x", [B_SH, C, L], _F32, kind="ExternalInput")
    alpha = nc.dram_tensor("alpha", [1, C], _F32, kind="ExternalInput")
    y = nc.dram_tensor("y", [B_SH, C, L], _F32, kind="ExternalOutput")

    mult = mybir.AluOpType.mult
    add = mybir.AluOpType.add

    with TileContext(nc) as tc:
        with (
            tc.tile_pool(name="xp", bufs=4) as xp,
            tc.tile_pool(name="bp", bufs=4) as bp,
            tc.tile_pool(name="yp", bufs=4) as yp,
            tc.tile_pool(name="cp", bufs=1) as cp,
        ):
            ones = cp.tile([P, L], _F32, tag="ones", name="ones")
            nc.vector.memset(ones, 1.0)

            a_col = []
            d_bc = []
            for cb in range(N_CB):
                a_t = cp.tile([P, 1], _F32, tag=f"a{cb}", name=f"a{cb}")
                nc.sync.dma_start(
                    out=a_t,
                    in_=alpha[0:1, cb * P : (cb + 1) * P].rearrange(
                        "o (p b) -> (o p) b", b=1
                    ),
                )
                d_t = cp.tile([P, 1], _F32, tag=f"dt{cb}", name=f"dt{cb}")
                nc.vector.tensor_scalar(
                    out=d_t, in0=a_t, scalar1=-1.0, scalar2=1.0, op0=mult, op1=add
                )
                # d broadcast along the free dim for the scan's data0 stream
                d_b = cp.tile([P, L], _F32, tag=f"d{cb}", name=f"d{cb}")
                nc.scalar.mul(d_b, ones, d_t[:, 0:1])
                a_col.append(a_t)
                d_bc.append(d_b)

            for cb in range(N_CB):
                cs = slice(cb * P, (cb + 1) * P)
                for b in range(B_SH):
                    xt = xp.tile([P, L], _F32, tag="x", name="xt")
                    nc.sync.dma_start(out=xt, in_=x[b, cs, :])

                    bt = bp.tile([P, L], _F32, tag="b", name="bt")
                    nc.scalar.mul(bt, xt, a_col[cb][:, 0:1])

                    yt = yp.tile([P, L], _F32, tag="y", name="yt")
                    nc.vector.tensor_tensor_scan(
                        out=yt[:, 1:],
                        data0=d_bc[cb][:, 1:],
                        data1=bt[:, 1:],
                        initial=xt[:, 0:1],
                        op0=mult,
                        op1=add,
                    )
                    nc.vector.tensor_copy(yt[:, 0:1], xt[:, 0:1])

                    nc.sync.dma_start(out=y[b, cs, :], in_=yt)

    nc.compile()
    return nc


_cached_nc = None


def _get_nc() -> bass.Bass:
    global _cached_nc
    if _cached_nc is None:
        _cached_nc = build_nc()
    return _cached_nc


def kernel(x: np.ndarray, alpha: np.ndarray) -> np.ndarray:
    assert x.shape == (B, C, L) and alpha.shape == (1, C)
    x = np.ascontiguousarray(x, dtype=np.float32)
    alpha = np.ascontiguousarray(alpha, dtype=np.float32)
    nc = _get_nc()
    in_maps = [
        {"x": x[c * B_SH : (c + 1) * B_SH], "alpha": alpha} for c in range(N_CORES)
    ]
    res = run_bass_kernel_spmd(nc, in_maps, list(range(N_CORES)))
    return np.concatenate([r["y"] for r in res.results], axis=0)


# revision 5
# speedup vs baseline: 1.2490x; 1.2490x over previous
"""Per-channel EMA (first-order linear recurrence along time) on 8 TRN2 cores.

  y[b, c, 0] = x[b, c, 0]
  y[b, c, t] = (1 - alpha[c]) * y[b, c, t-1] + alpha[c] * x[b, c, t]

Strategy
  - Data-parallel over batch: B=32 -> 4 batches per core, alpha replicated.
  - Per core: 16 tiles of [128 channels (partitions), 2048 time (free)].
  - The recurrence runs on the DVE via tensor_tensor_scan:
        state = (d * state) + a*x_t,   d = 1 - alpha (per partition)
    with initial = x[:, 0] as a per-partition AP. Column 0 needs no special
    case: d*x0 + a*x0 = x0.
  - The alpha pre-scale (a*x) runs on the Scalar/ACT engine; both compute
    passes hide behind the HBM DMA (memory bound: 32 MiB per core round trip).
  - TRN2 has two physical HWDGE rings that each pop DMA triggers in FIFO
    order: qSPDynamicHW (sync) and qActDynamicHW (scalar). Loads go on the
    sync ring (their waits are stale slot-reuse checks) and stores on the
    scalar ring (their waits are data-ready and would otherwise block loads).
"""

import numpy as np

import concourse.bass as bass
import concourse.bacc as bacc
import concourse.mybir as mybir
from concourse.tile import TileContext
from concourse.bass_utils import run_bass_kernel_spmd

B, C, L = 32, 512, 2048
N_CORES = 8
B_SH = B // N_CORES  # 4 batches per core
P = 128              # SBUF partitions
N_CB = C // P        # 4 channel blocks

_F32 = mybir.dt.float32


def build_nc() -> bass.Bass:
    # Bacc (not raw Bass): its compile() runs generate_event_semaphores,
    # which splits multi-sem waits — TRN2 allows at most one wait command
    # per instruction, and Tile freely emits several.
    nc = bacc.Bacc()
    x = nc.dram_tensor("x", [B_SH, C, L], _F32, kind="ExternalInput")
    alpha = nc.dram_tensor("alpha", [1, C], _F32, kind="ExternalInput")
    y = nc.dram_tensor("y", [B_SH, C, L], _F32, kind="ExternalOutput")

    mult = mybir.AluOpType.mult
    add = mybir.AluOpType.add

    with TileContext(nc) as tc:
        with (
            tc.tile_pool(name="xp", bufs=6) as xp,
            tc.tile_pool(name="bp", bufs=4) as bp,
            tc.tile_pool(name="yp", bufs=4) as yp,
            tc.tile_pool(name="cp", bufs=1) as cp,
        ):
            ones = cp.tile([P, L], _F32, tag="ones", name="ones")
            nc.gpsimd.memset(ones, 1.0)

            a_col = []
            d_bc = []
            for cb in range(N_CB):
                a_t = cp.tile([P, 1], _F32, tag=f"a{cb}", name=f"a{cb}")
                nc.sync.dma_start(
                    out=a_t,
                    in_=alpha[0:1, cb * P : (cb + 1) * P].rearrange(
                        "o (p b) -> (o p) b", b=1
                    ),
                )
                d_t = cp.tile([P, 1], _F32, tag=f"dt{cb}", name=f"dt{cb}")
                nc.vector.tensor_scalar(
                    out=d_t, in0=a_t, scalar1=-1.0, scalar2=1.0, op0=mult, op1=add
                )
                # d broadcast along the free dim for the scan's data0 stream
                d_b = cp.tile([P, L], _F32, tag=f"d{cb}", name=f"d{cb}")
                nc.scalar.mul(d_b, ones, d_t[:, 0:1])
                a_col.append(a_t)
                d_bc.append(d_b)

            for cb in range(N_CB):
                cs = slice(cb * P, (cb + 1) * P)
                for b in range(B_SH):
                    xt = xp.tile([P, L], _F32, tag="x", name="xt")
                    nc.sync.dma_start(out=xt, in_=x[b, cs, :])

                    bt = bp.tile([P, L], _F32, tag="b", name="bt")
                    nc.scalar.mul(bt, xt, a_col[cb][:, 0:1])

                    yt = yp.tile([P, L], _F32, tag="y", name="yt")
                    nc.vector.tensor_tensor_scan(
                        out=yt,
                        data0=d_bc[cb],
                        data1=bt,
                        initial=xt[:, 0:1],
                        op0=mult,
                        op1=add,
                    )
                    nc.scalar.dma_start(out=y[b, cs, :], in_=yt)

    nc.compile()
    return nc


_cached_nc = None


def _get_nc() -> bass.Bass:
    global _cached_nc
    if _cached_nc is None:
        _cached_nc = build_nc()
    return _cached_nc


def kernel(x: np.ndarray, alpha: np.ndarray) -> np.ndarray:
    assert x.shape == (B, C, L) and alpha.shape == (1, C)
    x = np.ascontiguousarray(x, dtype=np.float32)
    alpha = np.ascontiguousarray(alpha, dtype=np.float32)
    nc = _get_nc()
    in_maps = [
        {"x": x[c * B_SH : (c + 1) * B_SH], "alpha": alpha} for c in range(N_CORES)
    ]
    res = run_bass_kernel_spmd(nc, in_maps, list(range(N_CORES)))
    return np.concatenate([r["y"] for r in res.results], axis=0)


# revision 7
# speedup vs baseline: 1.3054x; 1.0452x over previous
"""Per-channel EMA (first-order linear recurrence along time) on 8 TRN2 cores.

  y[b, c, 0] = x[b, c, 0]
  y[b, c, t] = (1 - alpha[c]) * y[b, c, t-1] + alpha[c] * x[b, c, t]

Strategy
  - Data-parallel over batch: B=32 -> 4 batches per core, alpha replicated.
  - Per core: 16 tiles of [128 channels (partitions), 2048 time (free)].
  - The recurrence runs on the DVE via tensor_tensor_scan:
        state = (d * state) + a*x_t,   d = 1 - alpha (per partition)
    with initial = x[:, 0] as a per-partition AP. Column 0 needs no special
    case: d*x0 + a*x0 = x0.
  - The alpha pre-scale (a*x) runs on the Scalar/ACT engine; both compute
    passes hide behind the HBM DMA (memory bound: 32 MiB per core round trip).
  - Engine-queue discipline (Tile emits conservative producer-queue waits, so
    a consumer ends up waiting for *everything* scheduled earlier on that
    queue): loads go on the sync queue (stale slot-reuse waits only), stores
    on the otherwise-idle PE queue (their data-ready waits then block nothing
    else), and the ACT queue carries only the prescales so it always runs
    ahead of the DVE scan chain.
"""

import numpy as np

import concourse.bass as bass
import concourse.bacc as bacc
import concourse.mybir as mybir
from concourse.tile import TileContext
from concourse.bass_utils import run_bass_kernel_spmd

B, C, L = 32, 512, 2048
N_CORES = 8
B_SH = B // N_CORES  # 4 batches per core
P = 128              # SBUF partitions
N_CB = C // P        # 4 channel blocks

_F32 = mybir.dt.float32

# data0 of the scan as a stride-0 broadcast AP over the [P, 1] decay column
# instead of a materialized [P, L] tile.
BCAST_D = True


def build_nc() -> bass.Bass:
    # Bacc (not raw Bass): its compile() runs generate_event_semaphores,
    # which splits multi-sem waits — TRN2 allows at most one wait command
    # per instruction, and Tile freely emits several.
    nc = bacc.Bacc()
    x = nc.dram_tensor("x", [B_SH, C, L], _F32, kind="ExternalInput")
    alpha = nc.dram_tensor("alpha", [1, C], _F32, kind="ExternalInput")
    y = nc.dram_tensor("y", [B_SH, C, L], _F32, kind="ExternalOutput")

    mult = mybir.AluOpType.mult
    add = mybir.AluOpType.add

    with TileContext(nc) as tc:
        with (
            tc.tile_pool(name="xp", bufs=6) as xp,
            tc.tile_pool(name="bp", bufs=6) as bp,
            tc.tile_pool(name="yp", bufs=5) as yp,
            tc.tile_pool(name="cp", bufs=1) as cp,
        ):
            # all 4 channel blocks of alpha in one DMA: [P, N_CB], col j =
            # alpha[j*P + p]
            a4 = cp.tile([P, N_CB], _F32, tag="a4", name="a4")
            nc.sync.dma_start(out=a4, in_=alpha[0].rearrange("(j p) -> p j", j=N_CB))
            d4 = cp.tile([P, N_CB], _F32, tag="d4", name="d4")
            nc.vector.tensor_scalar(
                out=d4, in0=a4, scalar1=-1.0, scalar2=1.0, op0=mult, op1=add
            )

            d_bc = []
            if not BCAST_D:
                ones = cp.tile([P, L], _F32, tag="ones", name="ones")
                nc.gpsimd.memset(ones, 1.0)
                for cb in range(N_CB):
                    d_b = cp.tile([P, L], _F32, tag=f"d{cb}", name=f"d{cb}")
                    nc.scalar.mul(d_b, ones, d4[:, cb : cb + 1])
                    d_bc.append(d_b)

            for cb in range(N_CB):
                cs = slice(cb * P, (cb + 1) * P)
                if BCAST_D:
                    d_stream = d4[:, cb : cb + 1].broadcast_to([P, L])
                else:
                    d_stream = d_bc[cb]
                for b in range(B_SH):
                    xt = xp.tile([P, L], _F32, tag="x", name="xt")
                    nc.sync.dma_start(out=xt, in_=x[b, cs, :])

                    bt = bp.tile([P, L], _F32, tag="b", name="bt")
                    nc.scalar.mul(bt, xt, a4[:, cb : cb + 1])

                    yt = yp.tile([P, L], _F32, tag="y", name="yt")
                    nc.vector.tensor_tensor_scan(
                        out=yt,
                        data0=d_stream,
                        data1=bt,
                        initial=xt[:, 0:1],
                        op0=mult,
                        op1=add,
                    )
                    nc.gpsimd.dma_start(out=y[b, cs, :], in_=yt)

    nc.compile()
    return nc


_cached_nc = None


def _get_nc() -> bass.Bass:
    global _cached_nc
    if _cached_nc is None:
        _cached_nc = build_nc()
    return _cached_nc


def kernel(x: np.ndarray, alpha: np.ndarray) -> np.ndarray:
    assert x.shape == (B, C, L) and alpha.shape == (1, C)
    x = np.ascontiguousarray(x, dtype=np.float32)
    alpha = np.ascontiguousarray(alpha, dtype=np.float32)
    nc = _get_nc()
    in_maps = [
        {"x": x[c * B_SH : (c + 1) * B_SH], "alpha": alpha} for c in range(N_CORES)
    ]
    res = run_bass_kernel_spmd(nc, in_maps, list(range(N_CORES)))
    return np.concatenate([r["y"] for r in res.results], axis=0)
